# revision 22
# baseline (speedup 1.0000x reference)
"""DeltaGN message-passing kernel for 8 Trainium2 NeuronCores.

Strategy (fully data/graph parallel, no collectives needed):
  The reference aggregation is `En.reshape(B, N, E//N, ED).sum(2)` -- a
  block-local reshape-sum: edge e contributes to node e // 64.  So sharding
  edges in aligned blocks of 64*512 gives each core 512 nodes (per batch)
  plus exactly the 32768 edges that aggregate into them.  V (tiny) is
  replicated so the per-edge endpoint gather V[R_s]/V[R_r] is core-local.

On-device layout:
  - Gather via GPSIMD ap_gather: per 16-partition group one index list.
    Partitions 32G+0..15  <- src endpoint features of edge-chunk G
    Partitions 32G+16..31 <- rcv endpoint features of the same chunk
    feature rows per sub-group: [mass, velx, vely, posx, posy, junk...]
  - Edge MLP layer 1 as a single K=21 matmul per 32-partition super-group
    (junk rows get zero weights), plus an accumulating K=5 matmul that adds
    the periodic-boundary wrap correction +-6 computed on DVE.
  - relu/bias fused into PSUM->SBUF evictions (DVE layer1, ACT layer2).
  - Aggregation: DVE tensor_reduce over 64 consecutive edge columns.
  - Node MLP + final linear + PBC wrap on-device; output [B, 4, 512]/core.
"""

import os
import numpy as np

B, N, E, F = 2, 4096, 262144, 5
BOX, HALF = 6.0, 3.0
ED, ND = 150, 100
NCORES = 8
NPC = N // NCORES            # nodes per core per batch   (512)
EPC = E // NCORES            # edges per core per batch   (32768)
NSUB = 4                     # gather sub-rounds per batch
SUBE = EPC // NSUB           # edges per sub-round        (8192)
GL = SUBE // 4               # cols per super-group       (2048)
NCH = 1024                   # eviction chunk (free dim)
EDGES_PER_NODE = E // N      # 64

_CACHE = {}


def _build_program():
    import concourse.bass as bass
    import concourse.tile as tile
    from concourse import bacc, mybir

    f32 = mybir.dt.float32
    i16 = mybir.dt.int16
    AF = mybir.ActivationFunctionType
    ALU = mybir.AluOpType

    nc = bacc.Bacc("TRN2", target_bir_lowering=False, debug=False,
                   num_devices=NCORES)

    def din(name, shape, dtype=f32):
        return nc.dram_tensor(name, list(shape), dtype, kind="ExternalInput").ap()

    # ---- DRAM inputs (host-prepacked, per core) ----
    D_hbm = din("D", (B, 128, N))                 # gather data rows
    IDX_hbm = din("IDX", (B, NSUB, 128, GL // 16), i16)
    WT_hbm = din("WT", (128, ED))                 # stacked K=21 edge-W
    WC_hbm = din("WC", (128, ED))                 # stacked K=5 corr-W
    W2A_hbm = din("W2A", (128, ED))
    W2B_hbm = din("W2B", (22, ED))
    B1_hbm = din("B1", (B, 128, 1))               # edge bias rows 0:128
    B1T_hbm = din("B1T", (B, 22, 1))              # edge bias rows 128:150
    B2_hbm = din("B2", (128, 1))
    B2T_hbm = din("B2T", (22, 1))
    VNP_hbm = din("VNP", (B, 3, NPC))             # node non_pos rows
    VBASE_hbm = din("VBASE", (B, 4, NPC))         # V[:,:,1:5] rows
    NW1NP_hbm = din("NW1NP", (3, ND))
    NW1A_hbm = din("NW1A", (128, ND))
    NW1B_hbm = din("NW1B", (22, ND))
    NW2_hbm = din("NW2", (ND, ND))
    NW3_hbm = din("NW3", (ND, ND))
    LW_hbm = din("LW", (ND, 4))
    THRH_hbm = din("THRH", (4, 1))                # [3,3,big,big]
    THRL_hbm = din("THRL", (4, 1))                # [-3,-3,-big,-big]
    NB1_hbm = din("NB1", (B, ND, 1))              # dt folded
    NB2_hbm = din("NB2", (ND, 1))
    NB3_hbm = din("NB3", (ND, 1))
    LB_hbm = din("LB", (4, 1))
    OUT_hbm = nc.dram_tensor("OUT", [B, 4, NPC], f32,
                             kind="ExternalOutput").ap()

    from contextlib import ExitStack
    with tile.TileContext(nc) as tc, ExitStack() as ctx:
        cpool = ctx.enter_context(tc.tile_pool(name="consts", bufs=1))
        dpool = ctx.enter_context(tc.tile_pool(name="dtile", bufs=2))
        ipool = ctx.enter_context(tc.tile_pool(name="idx", bufs=3))
        gpool = ctx.enter_context(tc.tile_pool(name="gath", bufs=2))
        spool1 = ctx.enter_context(tc.tile_pool(name="scr1", bufs=1))
        spool2 = ctx.enter_context(tc.tile_pool(name="scr2", bufs=1))
        spool3 = ctx.enter_context(tc.tile_pool(name="scr3", bufs=2))
        h1pool = ctx.enter_context(tc.tile_pool(name="h1", bufs=2))
        enpool = ctx.enter_context(tc.tile_pool(name="en", bufs=2))
        aggpool = ctx.enter_context(tc.tile_pool(name="agg", bufs=2))
        npool = ctx.enter_context(tc.tile_pool(name="node", bufs=2))
        ps1a = ctx.enter_context(tc.tile_pool(name="ps1a", bufs=1, space="PSUM"))
        ps1b = ctx.enter_context(tc.tile_pool(name="ps1b", bufs=1, space="PSUM"))
        ps2a = ctx.enter_context(tc.tile_pool(name="ps2a", bufs=1, space="PSUM"))
        ps2b = ctx.enter_context(tc.tile_pool(name="ps2b", bufs=1, space="PSUM"))

        # ---- load constants ----
        _cn = [0]

        def const(src, shape, dtype=f32):
            _cn[0] += 1
            t = cpool.tile(list(shape), dtype, tag=f"c{_cn[0]}")
            nc.sync.dma_start(t[...], src)
            return t

        WT = const(WT_hbm[...], (128, ED))
        WC = const(WC_hbm[...], (128, ED))
        W2A = const(W2A_hbm[...], (128, ED))
        W2B = const(W2B_hbm[...], (22, ED))
        B1 = [const(B1_hbm[b], (128, 1)) for b in range(B)]
        B1T = [const(B1T_hbm[b], (22, 1)) for b in range(B)]
        B2 = const(B2_hbm[...], (128, 1))
        B2T = const(B2T_hbm[...], (22, 1))
        NW1NP = const(NW1NP_hbm[...], (3, ND))
        NW1A = const(NW1A_hbm[...], (128, ND))
        NW1B = const(NW1B_hbm[...], (22, ND))
        NW2 = const(NW2_hbm[...], (ND, ND))
        NW3 = const(NW3_hbm[...], (ND, ND))
        LW = const(LW_hbm[...], (ND, 4))
        NB1 = [const(NB1_hbm[b], (ND, 1)) for b in range(B)]
        NB2 = const(NB2_hbm[...], (ND, 1))
        NB3 = const(NB3_hbm[...], (ND, 1))
        LB = const(LB_hbm[...], (4, 1))
        THRH = const(THRH_hbm[...], (4, 1))
        THRL = const(THRL_hbm[...], (4, 1))
        VNP = [const(VNP_hbm[b], (3, NPC)) for b in range(B)]
        VBASE = [const(VBASE_hbm[b], (4, NPC)) for b in range(B)]

        for b in range(B):
            Dt = dpool.tile([128, N], f32)
            nc.sync.dma_start(Dt[...], D_hbm[b])

            agg_a = aggpool.tile([128, NPC], f32, tag="agg_a")
            agg_b = aggpool.tile([22, NPC], f32, tag="agg_b")

            for r in range(NSUB):
                idx = ipool.tile([128, GL // 16], i16)
                nc.sync.dma_start(idx[...], IDX_hbm[b, r])

                GT = gpool.tile([128, GL], f32)
                nc.gpsimd.ap_gather(
                    GT[...], Dt[...], idx[...],
                    channels=128, num_elems=N, d=1, num_idxs=GL,
                )

                # wrap correction: corr = (d_pre <= -3) - (d_pre > 3),
                # d_pre[32G+3,4] = src_pos - rcv_pos (junk elsewhere).
                # rcv pos rows are DMA-shifted onto the src-aligned
                # partitions so the DVE ops can run full-width at base 0.
                scS = spool1.tile([128, GL], f32, tag="scS")
                for G in range(4):
                    nc.sync.dma_start(scS[32 * G + 3:32 * G + 5, :],
                                      GT[32 * G + 19:32 * G + 21, :])
                sc1 = spool1.tile([128, GL], f32, tag="sc1")
                sc2 = spool2.tile([128, GL], f32)
                sc3 = spool3.tile([128, GL], f32)
                nc.vector.tensor_sub(sc1[...], GT[...], scS[...])
                nc.vector.tensor_scalar(sc2[...], sc1[...],
                                        HALF, None, ALU.is_gt)
                nc.vector.scalar_tensor_tensor(
                    sc3[...], sc1[...], -HALF, sc2[...],
                    ALU.is_le, ALU.subtract)

                for G in range(4):
                    p0 = 32 * G
                    En_a = enpool.tile([128, GL], f32, tag="en_a")
                    En_b = enpool.tile([22, GL], f32, tag="en_b")
                    for nch in range(GL // NCH):
                        c0 = nch * NCH
                        p1a = ps1a.tile([128, NCH], f32, tag='ps1a')
                        p1b = ps1b.tile([22, NCH], f32, tag='ps1b')
                        for h in range(NCH // 512):
                            cs = slice(c0 + h * 512, c0 + (h + 1) * 512)
                            hs = slice(h * 512, (h + 1) * 512)
                            for (ps, ms) in ((p1a, slice(0, 128)),
                                             (p1b, slice(128, ED))):
                                nc.tensor.matmul(
                                    ps[:, hs], lhsT=WT[p0:p0 + 21, ms],
                                    rhs=GT[p0:p0 + 21, cs],
                                    start=True, stop=False,
                                    tile_position=(p0, 0))
                                nc.tensor.matmul(
                                    ps[:, hs], lhsT=WC[p0:p0 + 5, ms],
                                    rhs=sc3[p0:p0 + 5, cs],
                                    start=False, stop=True,
                                    tile_position=(p0, 0))
                        h1a = h1pool.tile([128, NCH], f32, tag="h1a")
                        h1b = h1pool.tile([22, NCH], f32, tag="h1b")
                        nc.vector.tensor_scalar(h1a[...], p1a[...],
                                                B1[b][0:128, :], 0.0,
                                                ALU.add, ALU.max)
                        nc.vector.tensor_scalar(h1b[...], p1b[...],
                                                B1T[b][...], 0.0,
                                                ALU.add, ALU.max)
                        p2a = ps2a.tile([128, NCH], f32, tag='ps2a')
                        p2b = ps2b.tile([22, NCH], f32, tag='ps2b')
                        for h in range(NCH // 512):
                            hs = slice(h * 512, (h + 1) * 512)
                            for (ps, ms) in ((p2a, slice(0, 128)),
                                             (p2b, slice(128, ED))):
                                nc.tensor.matmul(
                                    ps[:, hs], lhsT=W2A[:, ms],
                                    rhs=h1a[:, hs], start=True, stop=False)
                                nc.tensor.matmul(
                                    ps[:, hs], lhsT=W2B[:, ms],
                                    rhs=h1b[:, hs], start=False, stop=True)
                        nc.scalar.activation(En_a[:, c0:c0 + NCH], p2a[...],
                                             AF.Relu, bias=B2[0:128, :])
                        nc.scalar.activation(En_b[:, c0:c0 + NCH], p2b[...],
                                             AF.Relu, bias=B2T[...])
                    n0 = 128 * G + 32 * r
                    nc.vector.tensor_reduce(
                        agg_a[:, n0:n0 + 32],
                        En_a[...].rearrange("p (n s) -> p n s", s=64),
                        mybir.AxisListType.X, ALU.add)
                    nc.vector.tensor_reduce(
                        agg_b[:, n0:n0 + 32],
                        En_b[...].rearrange("p (n s) -> p n s", s=64),
                        mybir.AxisListType.X, ALU.add)

            # ---- node MLP ----
            pn = ps1a.tile([ND, NPC], f32, tag='ps1a')
            nc.tensor.matmul(pn[...], lhsT=NW1NP[...], rhs=VNP[b][...],
                             start=True, stop=False)
            nc.tensor.matmul(pn[...], lhsT=NW1A[...], rhs=agg_a[...],
                             start=False, stop=False)
            nc.tensor.matmul(pn[...], lhsT=NW1B[...], rhs=agg_b[...],
                             start=False, stop=True)
            hn1 = npool.tile([ND, NPC], f32, tag="hn")
            nc.vector.tensor_scalar(hn1[...], pn[...], NB1[b][...], 0.0,
                                    ALU.add, ALU.max)
            pn2 = ps2a.tile([ND, NPC], f32, tag='ps2a')
            nc.tensor.matmul(pn2[...], lhsT=NW2[...], rhs=hn1[...],
                             start=True, stop=True)
            hn2 = npool.tile([ND, NPC], f32, tag="hn")
            nc.vector.tensor_scalar(hn2[...], pn2[...], NB2[...], 0.0,
                                    ALU.add, ALU.max)
            pn3 = ps1b.tile([ND, NPC], f32, tag='ps1b')
            nc.tensor.matmul(pn3[...], lhsT=NW3[...], rhs=hn2[...],
                             start=True, stop=True)
            hn3 = npool.tile([ND, NPC], f32, tag="hn")
            nc.vector.tensor_scalar(hn3[...], pn3[...], NB3[...], 0.0,
                                    ALU.add, ALU.max)
            pf = ps2b.tile([4, NPC], f32, tag='ps2b')
            nc.tensor.matmul(pf[...], lhsT=LW[...], rhs=hn3[...],
                             start=True, stop=True)
            delta = npool.tile([4, NPC], f32, tag="delta")
            nc.scalar.activation(delta[...], pf[...], AF.Identity,
                                 bias=LB[...])
            newv = npool.tile([4, NPC], f32, tag="newv")
            nc.vector.tensor_add(newv[...], delta[...], VBASE[b][...])
            tA = npool.tile([4, NPC], f32, tag="tA")
            nc.vector.tensor_scalar(tA[...], newv[...], THRH[...], None,
                                    ALU.is_ge)
            tB = npool.tile([4, NPC], f32, tag="tB")
            nc.vector.scalar_tensor_tensor(tB[...], newv[...], THRL[...],
                                           tA[...], ALU.is_lt, ALU.subtract)
            outc = npool.tile([4, NPC], f32, tag="outc")
            nc.vector.scalar_tensor_tensor(outc[...], tB[...], BOX,
                                           newv[...], ALU.mult, ALU.add)
            nc.sync.dma_start(OUT_hbm[b], outc[...])

    nc.compile()
    return nc


def _pack_inputs(V, dt, eW1, eb1, eW2, eb2, nW1, nb1, nW2, nb2, nW3, nb3,
                 lW, lb, R_s, R_r):
    """Build the per-core input dicts (host-side preprocessing)."""
    V = np.asarray(V, np.float32)
    dt = np.asarray(dt, np.float32)
    eW1 = np.asarray(eW1, np.float32)
    eW2 = np.asarray(eW2, np.float32)
    nW1 = np.asarray(nW1, np.float32)
    R_s = np.asarray(R_s).astype(np.int64)
    R_r = np.asarray(R_r).astype(np.int64)

    fmap = [0, 3, 4, 1, 2] + [0] * 11
    Vt = np.transpose(V, (0, 2, 1))                    # [B, 5, N]
    D = np.empty((B, 128, N), np.float32)
    for s in range(8):                                  # 8 sub-groups of 16
        D[:, 16 * s:16 * (s + 1), :] = Vt[:, fmap, :]

    # W~ [21 rows] stacked at the four 32-partition bases
    WTs = np.zeros((128, ED), np.float32)
    WCs = np.zeros((128, ED), np.float32)
    w21 = np.zeros((21, ED), np.float32)
    w21[0:3] = eW1[0:3]
    w21[3:5] = eW1[6:8]
    w21[16:19] = eW1[3:6]
    w21[19:21] = -eW1[6:8]
    for G in range(4):
        WTs[32 * G:32 * G + 21] = w21
        WCs[32 * G + 3] = BOX * eW1[6]
        WCs[32 * G + 4] = BOX * eW1[7]

    W2A = np.ascontiguousarray(eW2[0:128])
    W2B = np.ascontiguousarray(eW2[128:150])
    B1 = np.zeros((B, 128, 1), np.float32)
    B1T = np.zeros((B, 22, 1), np.float32)
    for b in range(B):
        bb = np.asarray(eb1, np.float32) + dt[b, 0] * eW1[8]
        B1[b, :, 0] = bb[0:128]
        B1T[b, :, 0] = bb[128:150]
    eb2f = np.asarray(eb2, np.float32)
    B2 = np.ascontiguousarray(eb2f[0:128]).reshape(128, 1)
    B2T = np.ascontiguousarray(eb2f[128:150]).reshape(22, 1)

    NW1NP = np.ascontiguousarray(nW1[0:3])
    NW1A = np.ascontiguousarray(nW1[3:131])
    NW1B = np.ascontiguousarray(nW1[131:153])
    NB1 = np.zeros((B, ND, 1), np.float32)
    for b in range(B):
        NB1[b, :, 0] = np.asarray(nb1, np.float32) + dt[b, 0] * nW1[153]
    BIG = np.float32(1e30)
    THRH = np.array([[HALF], [HALF], [BIG], [BIG]], np.float32)
    THRL = np.array([[-HALF], [-HALF], [-BIG], [-BIG]], np.float32)
    NB2 = np.asarray(nb2, np.float32).reshape(ND, 1)
    NB3 = np.asarray(nb3, np.float32).reshape(ND, 1)
    LB = np.asarray(lb, np.float32).reshape(4, 1)
    NW2 = np.asarray(nW2, np.float32)
    NW3 = np.asarray(nW3, np.float32)
    LW = np.asarray(lW, np.float32)

    in_maps = []
    for c in range(NCORES):
        nodes = slice(NPC * c, NPC * (c + 1))
        IDX = np.empty((B, NSUB, 128, GL // 16), np.int16)
        for b in range(B):
            for r in range(NSUB):
                for G in range(4):
                    e0 = EPC * c + SUBE * G + GL * r
                    ls = R_s[b, e0:e0 + GL].astype(np.int16)
                    lr = R_r[b, e0:e0 + GL].astype(np.int16)
                    IDX[b, r, 32 * G:32 * G + 16] = \
                        ls.reshape(GL // 16, 16).T
                    IDX[b, r, 32 * G + 16:32 * G + 32] = \
                        lr.reshape(GL // 16, 16).T
        VNP = np.ascontiguousarray(
            np.transpose(V[:, nodes][:, :, [0, 3, 4]], (0, 2, 1)))
        VBASE = np.ascontiguousarray(
            np.transpose(V[:, nodes][:, :, 1:5], (0, 2, 1)))
        in_maps.append({
            "D": D, "IDX": IDX, "WT": WTs, "WC": WCs,
            "W2A": W2A, "W2B": W2B, "B1": B1, "B1T": B1T,
            "B2": B2, "B2T": B2T,
            "VNP": VNP, "VBASE": VBASE,
            "NW1NP": NW1NP, "NW1A": NW1A, "NW1B": NW1B,
            "NW2": NW2, "NW3": NW3, "LW": LW,
            "NB1": NB1, "NB2": NB2, "NB3": NB3, "LB": LB,
            "THRH": THRH, "THRL": THRL,
        })
    return in_maps


def _get_runner():
    """Build (once) a cached jitted PJRT runner for the compiled program."""
    if "runner" in _CACHE:
        return _CACHE["runner"]
    import jax
    from jax.sharding import Mesh, PartitionSpec
    from jax.experimental.shard_map import shard_map
    from concourse import bass2jax, mybir

    if "nc" not in _CACHE:
        _CACHE["nc"] = _build_program()
    nc = _CACHE["nc"]
    bass2jax.install_neuronx_cc_hook()

    part_name = (nc.partition_id_tensor.name
                 if nc.partition_id_tensor else None)
    in_names, out_names, out_avals, zero_outs = [], [], [], []
    for alloc in nc.m.functions[0].allocations:
        if not isinstance(alloc, mybir.MemoryLocationSet):
            continue
        name = alloc.memorylocations[0].name
        if alloc.kind == "ExternalInput":
            if name != part_name:
                in_names.append(name)
        elif alloc.kind == "ExternalOutput":
            shape = tuple(alloc.tensor_shape)
            dtype = mybir.dt.np(alloc.dtype)
            out_names.append(name)
            out_avals.append(jax.core.ShapedArray(shape, dtype))
            zero_outs.append(np.zeros(shape, dtype))
    n_params = len(in_names)
    all_in_names = in_names + out_names
    if part_name is not None:
        all_in_names = all_in_names + [part_name]

    def _body(*args):
        operands = list(args)
        if part_name is not None:
            operands.append(bass2jax.partition_id_tensor())
        outs = bass2jax._bass_exec_p.bind(
            *operands,
            out_avals=tuple(out_avals),
            in_names=tuple(all_in_names),
            out_names=tuple(out_names),
            lowering_input_output_aliases=(),
            sim_require_finite=True,
            sim_require_nnan=True,
            nc=nc,
        )
        return tuple(outs)

    devices = jax.devices()[:NCORES]
    mesh = Mesh(np.asarray(devices), ("core",))
    n_outs = len(out_names)
    sharded = jax.jit(
        shard_map(
            _body, mesh=mesh,
            in_specs=(PartitionSpec("core"),) * (n_params + n_outs),
            out_specs=(PartitionSpec("core"),) * n_outs,
            check_rep=False,
        ),
        keep_unused=True,
    )
    runner = {
        "fn": sharded, "in_names": in_names, "out_names": out_names,
        "out_avals": out_avals, "zero_outs": zero_outs, "mesh": mesh,
    }
    _CACHE["runner"] = runner
    return runner


def _run(in_maps):
    rn = _get_runner()
    concat_in = [
        np.concatenate([m[name] for m in in_maps], axis=0)
        for name in rn["in_names"]
    ]
    concat_zeros = [
        np.zeros((NCORES * z.shape[0], *z.shape[1:]), z.dtype)
        for z in rn["zero_outs"]
    ]
    out_arrs = rn["fn"](*concat_in, *concat_zeros)
    return out_arrs


def kernel(**inputs):
    in_maps = _pack_inputs(**inputs)
    out_arrs = _run(in_maps)
    rn = _CACHE["runner"]
    i = rn["out_names"].index("OUT")
    o = np.asarray(out_arrs[i]).reshape(NCORES, B, 4, NPC)
    out = np.empty((B, N, 4), np.float32)
    for c in range(NCORES):
        out[:, NPC * c:NPC * (c + 1), :] = np.transpose(o[c], (0, 2, 1))
    return out


# revision 26
# speedup vs baseline: 27.0088x; 27.0088x over previous
"""DeltaGN message-passing kernel for 8 Trainium2 NeuronCores.

Strategy (fully data/graph parallel, no collectives needed):
  The reference aggregation is `En.reshape(B, N, E//N, ED).sum(2)` -- a
  block-local reshape-sum: edge e contributes to node e // 64.  So sharding
  edges in aligned blocks of 64*512 gives each core 512 nodes (per batch)
  plus exactly the 32768 edges that aggregate into them.  V (tiny) is
  replicated so the per-edge endpoint gather V[R_s]/V[R_r] is core-local.

On-device layout:
  - Gather via GPSIMD ap_gather: per 16-partition group one index list.
    Partitions 32G+0..15  <- src endpoint features of edge-chunk G
    Partitions 32G+16..31 <- rcv endpoint features of the same chunk
    feature rows per sub-group: [mass, velx, vely, posx, posy, junk...]
  - Edge MLP layer 1 as a single K=21 matmul per 32-partition super-group
    (junk rows get zero weights), plus an accumulating K=5 matmul that adds
    the periodic-boundary wrap correction +-6 computed on DVE.
  - relu/bias fused into PSUM->SBUF evictions (DVE layer1, ACT layer2).
  - Aggregation: DVE tensor_reduce over 64 consecutive edge columns.
  - Node MLP + final linear + PBC wrap on-device; output [B, 4, 512]/core.
"""

import os
import numpy as np

B, N, E, F = 2, 4096, 262144, 5
BOX, HALF = 6.0, 3.0
ED, ND = 150, 100
NCORES = 8
NPC = N // NCORES            # nodes per core per batch   (512)
EPC = E // NCORES            # edges per core per batch   (32768)
NSUB = 4                     # gather sub-rounds per batch
SUBE = EPC // NSUB           # edges per sub-round        (8192)
GL = SUBE // 4               # cols per super-group       (2048)
NCH = 1024                   # eviction chunk (free dim)
EDGES_PER_NODE = E // N      # 64

_CACHE = {}


def _build_program(repeat=1):
    import concourse.bass as bass
    import concourse.tile as tile
    from concourse import bacc, mybir

    f32 = mybir.dt.float32
    i16 = mybir.dt.int16
    AF = mybir.ActivationFunctionType
    ALU = mybir.AluOpType

    nc = bacc.Bacc("TRN2", target_bir_lowering=False, debug=False,
                   num_devices=NCORES)

    def din(name, shape, dtype=f32):
        return nc.dram_tensor(name, list(shape), dtype, kind="ExternalInput").ap()

    # ---- DRAM inputs (host-prepacked, per core) ----
    D_hbm = din("D", (B, 128, N))                 # gather data rows
    IDX_hbm = din("IDX", (B, NSUB, 128, GL // 16), i16)
    WT_hbm = din("WT", (128, ED))                 # stacked K=21 edge-W
    WC_hbm = din("WC", (128, ED))                 # stacked K=5 corr-W
    W2A_hbm = din("W2A", (128, ED))
    W2B_hbm = din("W2B", (22, ED))
    B1_hbm = din("B1", (B, 128, 1))               # edge bias rows 0:128
    B1T_hbm = din("B1T", (B, 22, 1))              # edge bias rows 128:150
    B2_hbm = din("B2", (128, 1))
    B2T_hbm = din("B2T", (22, 1))
    VNP_hbm = din("VNP", (B, 3, NPC))             # node non_pos rows
    VBASE_hbm = din("VBASE", (B, 4, NPC))         # V[:,:,1:5] rows
    NW1NP_hbm = din("NW1NP", (3, ND))
    NW1A_hbm = din("NW1A", (128, ND))
    NW1B_hbm = din("NW1B", (22, ND))
    NW2_hbm = din("NW2", (ND, ND))
    NW3_hbm = din("NW3", (ND, ND))
    LW_hbm = din("LW", (ND, 4))
    THRH_hbm = din("THRH", (4, 1))                # [3,3,big,big]
    THRL_hbm = din("THRL", (4, 1))                # [-3,-3,-big,-big]
    NB1_hbm = din("NB1", (B, ND, 1))              # dt folded
    NB2_hbm = din("NB2", (ND, 1))
    NB3_hbm = din("NB3", (ND, 1))
    LB_hbm = din("LB", (4, 1))
    OUT_hbm = nc.dram_tensor("OUT", [B, 4, NPC], f32,
                             kind="ExternalOutput").ap()

    from contextlib import ExitStack
    with tile.TileContext(nc) as tc, ExitStack() as ctx:
        cpool = ctx.enter_context(tc.tile_pool(name="consts", bufs=1))
        dpool = ctx.enter_context(tc.tile_pool(name="dtile", bufs=2))
        ipool = ctx.enter_context(tc.tile_pool(name="idx", bufs=3))
        gpool = ctx.enter_context(tc.tile_pool(name="gath", bufs=2))
        spool1 = ctx.enter_context(tc.tile_pool(name="scr1", bufs=1))
        spool2 = ctx.enter_context(tc.tile_pool(name="scr2", bufs=1))
        spool3 = ctx.enter_context(tc.tile_pool(name="scr3", bufs=2))
        h1pool = ctx.enter_context(tc.tile_pool(name="h1", bufs=2))
        enpool = ctx.enter_context(tc.tile_pool(name="en", bufs=2))
        aggpool = ctx.enter_context(tc.tile_pool(name="agg", bufs=2))
        npool = ctx.enter_context(tc.tile_pool(name="node", bufs=2))
        ps1a = ctx.enter_context(tc.tile_pool(name="ps1a", bufs=1, space="PSUM"))
        ps1b = ctx.enter_context(tc.tile_pool(name="ps1b", bufs=1, space="PSUM"))
        ps2a = ctx.enter_context(tc.tile_pool(name="ps2a", bufs=1, space="PSUM"))
        ps2b = ctx.enter_context(tc.tile_pool(name="ps2b", bufs=1, space="PSUM"))

        # ---- load constants ----
        _cn = [0]

        def const(src, shape, dtype=f32):
            _cn[0] += 1
            t = cpool.tile(list(shape), dtype, tag=f"c{_cn[0]}")
            nc.sync.dma_start(t[...], src)
            return t

        WT = const(WT_hbm[...], (128, ED))
        WC = const(WC_hbm[...], (128, ED))
        W2A = const(W2A_hbm[...], (128, ED))
        W2B = const(W2B_hbm[...], (22, ED))
        B1 = [const(B1_hbm[b], (128, 1)) for b in range(B)]
        B1T = [const(B1T_hbm[b], (22, 1)) for b in range(B)]
        B2 = const(B2_hbm[...], (128, 1))
        B2T = const(B2T_hbm[...], (22, 1))
        NW1NP = const(NW1NP_hbm[...], (3, ND))
        NW1A = const(NW1A_hbm[...], (128, ND))
        NW1B = const(NW1B_hbm[...], (22, ND))
        NW2 = const(NW2_hbm[...], (ND, ND))
        NW3 = const(NW3_hbm[...], (ND, ND))
        LW = const(LW_hbm[...], (ND, 4))
        NB1 = [const(NB1_hbm[b], (ND, 1)) for b in range(B)]
        NB2 = const(NB2_hbm[...], (ND, 1))
        NB3 = const(NB3_hbm[...], (ND, 1))
        LB = const(LB_hbm[...], (4, 1))
        THRH = const(THRH_hbm[...], (4, 1))
        THRL = const(THRL_hbm[...], (4, 1))
        VNP = [const(VNP_hbm[b], (3, NPC)) for b in range(B)]
        VBASE = [const(VBASE_hbm[b], (4, NPC)) for b in range(B)]

        for b in [bb for _ in range(repeat) for bb in range(B)]:
            Dt = dpool.tile([128, N], f32)
            nc.sync.dma_start(Dt[...], D_hbm[b])

            agg_a = aggpool.tile([128, NPC], f32, tag="agg_a")
            agg_b = aggpool.tile([22, NPC], f32, tag="agg_b")

            for r in range(NSUB):
                idx = ipool.tile([128, GL // 16], i16)
                nc.sync.dma_start(idx[...], IDX_hbm[b, r])

                GT = gpool.tile([128, GL], f32)
                nc.gpsimd.ap_gather(
                    GT[...], Dt[...], idx[...],
                    channels=128, num_elems=N, d=1, num_idxs=GL,
                )

                # wrap correction: corr = (d_pre <= -3) - (d_pre > 3),
                # d_pre[32G+3,4] = src_pos - rcv_pos (junk elsewhere).
                # rcv pos rows are DMA-shifted onto the src-aligned
                # partitions so the DVE ops can run full-width at base 0.
                scS = spool1.tile([128, GL], f32, tag="scS")
                for G in range(4):
                    nc.sync.dma_start(scS[32 * G + 3:32 * G + 5, :],
                                      GT[32 * G + 19:32 * G + 21, :])
                sc1 = spool1.tile([128, GL], f32, tag="sc1")
                sc2 = spool2.tile([128, GL], f32)
                sc3 = spool3.tile([128, GL], f32)
                nc.vector.tensor_sub(sc1[...], GT[...], scS[...])
                nc.vector.tensor_scalar(sc2[...], sc1[...],
                                        HALF, None, ALU.is_gt)
                nc.vector.scalar_tensor_tensor(
                    sc3[...], sc1[...], -HALF, sc2[...],
                    ALU.is_le, ALU.subtract)

                for G in range(4):
                    p0 = 32 * G
                    En_a = enpool.tile([128, GL], f32, tag="en_a")
                    En_b = enpool.tile([22, GL], f32, tag="en_b")
                    for nch in range(GL // NCH):
                        c0 = nch * NCH
                        p1a = ps1a.tile([128, NCH], f32, tag='ps1a')
                        p1b = ps1b.tile([22, NCH], f32, tag='ps1b')
                        for h in range(NCH // 512):
                            cs = slice(c0 + h * 512, c0 + (h + 1) * 512)
                            hs = slice(h * 512, (h + 1) * 512)
                            for (ps, ms) in ((p1a, slice(0, 128)),
                                             (p1b, slice(128, ED))):
                                nc.tensor.matmul(
                                    ps[:, hs], lhsT=WT[p0:p0 + 21, ms],
                                    rhs=GT[p0:p0 + 21, cs],
                                    start=True, stop=False,
                                    tile_position=(p0, 0))
                                nc.tensor.matmul(
                                    ps[:, hs], lhsT=WC[p0:p0 + 5, ms],
                                    rhs=sc3[p0:p0 + 5, cs],
                                    start=False, stop=True,
                                    tile_position=(p0, 0))
                        h1a = h1pool.tile([128, NCH], f32, tag="h1a")
                        h1b = h1pool.tile([22, NCH], f32, tag="h1b")
                        nc.vector.tensor_scalar(h1a[...], p1a[...],
                                                B1[b][0:128, :], 0.0,
                                                ALU.add, ALU.max)
                        nc.vector.tensor_scalar(h1b[...], p1b[...],
                                                B1T[b][...], 0.0,
                                                ALU.add, ALU.max)
                        p2a = ps2a.tile([128, NCH], f32, tag='ps2a')
                        p2b = ps2b.tile([22, NCH], f32, tag='ps2b')
                        for h in range(NCH // 512):
                            hs = slice(h * 512, (h + 1) * 512)
                            for (ps, ms) in ((p2a, slice(0, 128)),
                                             (p2b, slice(128, ED))):
                                nc.tensor.matmul(
                                    ps[:, hs], lhsT=W2A[:, ms],
                                    rhs=h1a[:, hs], start=True, stop=False)
                                nc.tensor.matmul(
                                    ps[:, hs], lhsT=W2B[:, ms],
                                    rhs=h1b[:, hs], start=False, stop=True)
                        nc.scalar.activation(En_a[:, c0:c0 + NCH], p2a[...],
                                             AF.Relu, bias=B2[0:128, :])
                        nc.scalar.activation(En_b[:, c0:c0 + NCH], p2b[...],
                                             AF.Relu, bias=B2T[...])
                    n0 = 128 * G + 32 * r
                    nc.vector.tensor_reduce(
                        agg_a[:, n0:n0 + 32],
                        En_a[...].rearrange("p (n s) -> p n s", s=64),
                        mybir.AxisListType.X, ALU.add)
                    nc.vector.tensor_reduce(
                        agg_b[:, n0:n0 + 32],
                        En_b[...].rearrange("p (n s) -> p n s", s=64),
                        mybir.AxisListType.X, ALU.add)

            # ---- node MLP ----
            pn = ps1a.tile([ND, NPC], f32, tag='ps1a')
            nc.tensor.matmul(pn[...], lhsT=NW1NP[...], rhs=VNP[b][...],
                             start=True, stop=False)
            nc.tensor.matmul(pn[...], lhsT=NW1A[...], rhs=agg_a[...],
                             start=False, stop=False)
            nc.tensor.matmul(pn[...], lhsT=NW1B[...], rhs=agg_b[...],
                             start=False, stop=True)
            hn1 = npool.tile([ND, NPC], f32, tag="hn")
            nc.vector.tensor_scalar(hn1[...], pn[...], NB1[b][...], 0.0,
                                    ALU.add, ALU.max)
            pn2 = ps2a.tile([ND, NPC], f32, tag='ps2a')
            nc.tensor.matmul(pn2[...], lhsT=NW2[...], rhs=hn1[...],
                             start=True, stop=True)
            hn2 = npool.tile([ND, NPC], f32, tag="hn")
            nc.vector.tensor_scalar(hn2[...], pn2[...], NB2[...], 0.0,
                                    ALU.add, ALU.max)
            pn3 = ps1b.tile([ND, NPC], f32, tag='ps1b')
            nc.tensor.matmul(pn3[...], lhsT=NW3[...], rhs=hn2[...],
                             start=True, stop=True)
            hn3 = npool.tile([ND, NPC], f32, tag="hn")
            nc.vector.tensor_scalar(hn3[...], pn3[...], NB3[...], 0.0,
                                    ALU.add, ALU.max)
            pf = ps2b.tile([4, NPC], f32, tag='ps2b')
            nc.tensor.matmul(pf[...], lhsT=LW[...], rhs=hn3[...],
                             start=True, stop=True)
            delta = npool.tile([4, NPC], f32, tag="delta")
            nc.scalar.activation(delta[...], pf[...], AF.Identity,
                                 bias=LB[...])
            newv = npool.tile([4, NPC], f32, tag="newv")
            nc.vector.tensor_add(newv[...], delta[...], VBASE[b][...])
            tA = npool.tile([4, NPC], f32, tag="tA")
            nc.vector.tensor_scalar(tA[...], newv[...], THRH[...], None,
                                    ALU.is_ge)
            tB = npool.tile([4, NPC], f32, tag="tB")
            nc.vector.scalar_tensor_tensor(tB[...], newv[...], THRL[...],
                                           tA[...], ALU.is_lt, ALU.subtract)
            outc = npool.tile([4, NPC], f32, tag="outc")
            nc.vector.scalar_tensor_tensor(outc[...], tB[...], BOX,
                                           newv[...], ALU.mult, ALU.add)
            nc.sync.dma_start(OUT_hbm[b], outc[...])

    nc.compile()
    return nc


def _pack_inputs(V, dt, eW1, eb1, eW2, eb2, nW1, nb1, nW2, nb2, nW3, nb3,
                 lW, lb, R_s, R_r):
    """Build the per-core input dicts (host-side preprocessing)."""
    V = np.asarray(V, np.float32)
    dt = np.asarray(dt, np.float32)
    eW1 = np.asarray(eW1, np.float32)
    eW2 = np.asarray(eW2, np.float32)
    nW1 = np.asarray(nW1, np.float32)
    R_s = np.asarray(R_s).astype(np.int64)
    R_r = np.asarray(R_r).astype(np.int64)

    fmap = [0, 3, 4, 1, 2] + [0] * 11
    Vt = np.transpose(V, (0, 2, 1))                    # [B, 5, N]
    D = np.empty((B, 128, N), np.float32)
    for s in range(8):                                  # 8 sub-groups of 16
        D[:, 16 * s:16 * (s + 1), :] = Vt[:, fmap, :]

    # W~ [21 rows] stacked at the four 32-partition bases
    WTs = np.zeros((128, ED), np.float32)
    WCs = np.zeros((128, ED), np.float32)
    w21 = np.zeros((21, ED), np.float32)
    w21[0:3] = eW1[0:3]
    w21[3:5] = eW1[6:8]
    w21[16:19] = eW1[3:6]
    w21[19:21] = -eW1[6:8]
    for G in range(4):
        WTs[32 * G:32 * G + 21] = w21
        WCs[32 * G + 3] = BOX * eW1[6]
        WCs[32 * G + 4] = BOX * eW1[7]

    W2A = np.ascontiguousarray(eW2[0:128])
    W2B = np.ascontiguousarray(eW2[128:150])
    B1 = np.zeros((B, 128, 1), np.float32)
    B1T = np.zeros((B, 22, 1), np.float32)
    for b in range(B):
        bb = np.asarray(eb1, np.float32) + dt[b, 0] * eW1[8]
        B1[b, :, 0] = bb[0:128]
        B1T[b, :, 0] = bb[128:150]
    eb2f = np.asarray(eb2, np.float32)
    B2 = np.ascontiguousarray(eb2f[0:128]).reshape(128, 1)
    B2T = np.ascontiguousarray(eb2f[128:150]).reshape(22, 1)

    NW1NP = np.ascontiguousarray(nW1[0:3])
    NW1A = np.ascontiguousarray(nW1[3:131])
    NW1B = np.ascontiguousarray(nW1[131:153])
    NB1 = np.zeros((B, ND, 1), np.float32)
    for b in range(B):
        NB1[b, :, 0] = np.asarray(nb1, np.float32) + dt[b, 0] * nW1[153]
    BIG = np.float32(1e30)
    THRH = np.array([[HALF], [HALF], [BIG], [BIG]], np.float32)
    THRL = np.array([[-HALF], [-HALF], [-BIG], [-BIG]], np.float32)
    NB2 = np.asarray(nb2, np.float32).reshape(ND, 1)
    NB3 = np.asarray(nb3, np.float32).reshape(ND, 1)
    LB = np.asarray(lb, np.float32).reshape(4, 1)
    NW2 = np.asarray(nW2, np.float32)
    NW3 = np.asarray(nW3, np.float32)
    LW = np.asarray(lW, np.float32)

    in_maps = []
    for c in range(NCORES):
        nodes = slice(NPC * c, NPC * (c + 1))
        IDX = np.empty((B, NSUB, 128, GL // 16), np.int16)
        for b in range(B):
            for r in range(NSUB):
                for G in range(4):
                    e0 = EPC * c + SUBE * G + GL * r
                    ls = R_s[b, e0:e0 + GL].astype(np.int16)
                    lr = R_r[b, e0:e0 + GL].astype(np.int16)
                    IDX[b, r, 32 * G:32 * G + 16] = \
                        ls.reshape(GL // 16, 16).T
                    IDX[b, r, 32 * G + 16:32 * G + 32] = \
                        lr.reshape(GL // 16, 16).T
        VNP = np.ascontiguousarray(
            np.transpose(V[:, nodes][:, :, [0, 3, 4]], (0, 2, 1)))
        VBASE = np.ascontiguousarray(
            np.transpose(V[:, nodes][:, :, 1:5], (0, 2, 1)))
        in_maps.append({
            "D": D, "IDX": IDX, "WT": WTs, "WC": WCs,
            "W2A": W2A, "W2B": W2B, "B1": B1, "B1T": B1T,
            "B2": B2, "B2T": B2T,
            "VNP": VNP, "VBASE": VBASE,
            "NW1NP": NW1NP, "NW1A": NW1A, "NW1B": NW1B,
            "NW2": NW2, "NW3": NW3, "LW": LW,
            "NB1": NB1, "NB2": NB2, "NB3": NB3, "LB": LB,
            "THRH": THRH, "THRL": THRL,
        })
    return in_maps


def _get_runner():
    """Build (once) a cached jitted PJRT runner for the compiled program."""
    if "runner" in _CACHE:
        return _CACHE["runner"]
    if "nc" not in _CACHE:
        _CACHE["nc"] = _build_program()
    _CACHE["runner"] = _make_runner(_CACHE["nc"])
    return _CACHE["runner"]


def _make_runner(nc):
    import jax
    from jax.sharding import Mesh, PartitionSpec
    from jax.experimental.shard_map import shard_map
    from concourse import bass2jax, mybir

    bass2jax.install_neuronx_cc_hook()

    part_name = (nc.partition_id_tensor.name
                 if nc.partition_id_tensor else None)
    in_names, out_names, out_avals, zero_outs = [], [], [], []
    for alloc in nc.m.functions[0].allocations:
        if not isinstance(alloc, mybir.MemoryLocationSet):
            continue
        name = alloc.memorylocations[0].name
        if alloc.kind == "ExternalInput":
            if name != part_name:
                in_names.append(name)
        elif alloc.kind == "ExternalOutput":
            shape = tuple(alloc.tensor_shape)
            dtype = mybir.dt.np(alloc.dtype)
            out_names.append(name)
            out_avals.append(jax.core.ShapedArray(shape, dtype))
            zero_outs.append(np.zeros(shape, dtype))
    n_params = len(in_names)
    all_in_names = in_names + out_names
    if part_name is not None:
        all_in_names = all_in_names + [part_name]

    def _body(*args):
        operands = list(args)
        if part_name is not None:
            operands.append(bass2jax.partition_id_tensor())
        outs = bass2jax._bass_exec_p.bind(
            *operands,
            out_avals=tuple(out_avals),
            in_names=tuple(all_in_names),
            out_names=tuple(out_names),
            lowering_input_output_aliases=(),
            sim_require_finite=True,
            sim_require_nnan=True,
            nc=nc,
        )
        return tuple(outs)

    devices = jax.devices()[:NCORES]
    mesh = Mesh(np.asarray(devices), ("core",))
    n_outs = len(out_names)
    sharded = jax.jit(
        shard_map(
            _body, mesh=mesh,
            in_specs=(PartitionSpec("core"),) * (n_params + n_outs),
            out_specs=(PartitionSpec("core"),) * n_outs,
            check_rep=False,
        ),
        keep_unused=True,
    )
    return {
        "fn": sharded, "in_names": in_names, "out_names": out_names,
        "out_avals": out_avals, "zero_outs": zero_outs, "mesh": mesh,
    }


def _run(in_maps):
    rn = _get_runner()
    concat_in = [
        np.concatenate([m[name] for m in in_maps], axis=0)
        for name in rn["in_names"]
    ]
    concat_zeros = [
        np.zeros((NCORES * z.shape[0], *z.shape[1:]), z.dtype)
        for z in rn["zero_outs"]
    ]
    out_arrs = rn["fn"](*concat_in, *concat_zeros)
    return out_arrs


def kernel(**inputs):
    in_maps = _pack_inputs(**inputs)
    out_arrs = _run(in_maps)
    rn = _CACHE["runner"]
    i = rn["out_names"].index("OUT")
    o = np.asarray(out_arrs[i]).reshape(NCORES, B, 4, NPC)
    out = np.empty((B, N, 4), np.float32)
    for c in range(NCORES):
        out[:, NPC * c:NPC * (c + 1), :] = np.transpose(o[c], (0, 2, 1))
    return out


# revision 33
# speedup vs baseline: 107.3658x; 3.9752x over previous
"""DeltaGN message-passing kernel for 8 Trainium2 NeuronCores.

Strategy (fully data/graph parallel, no collectives needed):
  The reference aggregation is `En.reshape(B, N, E//N, ED).sum(2)` -- a
  block-local reshape-sum: edge e contributes to node e // 64.  So sharding
  edges in aligned blocks of 64*512 gives each core 512 nodes (per batch)
  plus exactly the 32768 edges that aggregate into them.  V (tiny) is
  replicated so the per-edge endpoint gather V[R_s]/V[R_r] is core-local.

On-device layout:
  - Gather via GPSIMD ap_gather: per 16-partition group one index list.
    Partitions 32G+0..15  <- src endpoint features of edge-chunk G
    Partitions 32G+16..31 <- rcv endpoint features of the same chunk
    feature rows per sub-group: [mass, velx, vely, posx, posy, junk...]
  - Edge MLP layer 1 as a single K=21 matmul per 32-partition super-group
    (junk rows get zero weights), plus an accumulating K=5 matmul that adds
    the periodic-boundary wrap correction +-6 computed on DVE.
  - relu/bias fused into PSUM->SBUF evictions (DVE layer1, ACT layer2).
  - Aggregation: DVE tensor_reduce over 64 consecutive edge columns.
  - Node MLP + final linear + PBC wrap on-device; output [B, 4, 512]/core.
"""

import os
import numpy as np

B, N, E, F = 2, 4096, 262144, 5
BOX, HALF = 6.0, 3.0
ED, ND = 150, 100
NCORES = 8
NPC = N // NCORES            # nodes per core per batch   (512)
EPC = E // NCORES            # edges per core per batch   (32768)
NSUB = 4                     # gather sub-rounds per batch
SUBE = EPC // NSUB           # edges per sub-round        (8192)
GL = SUBE // 4               # cols per super-group       (2048)
NCH = 1024                   # eviction chunk (free dim)
EDGES_PER_NODE = E // N      # 64

_CACHE = {}


def _build_program(repeat=1, variant="full"):
    import concourse.bass as bass
    import concourse.tile as tile
    from concourse import bacc, mybir

    f32 = mybir.dt.float32
    bf16 = mybir.dt.bfloat16
    i16 = mybir.dt.int16
    AF = mybir.ActivationFunctionType
    ALU = mybir.AluOpType

    nc = bacc.Bacc("TRN2", target_bir_lowering=False, debug=False,
                   num_devices=NCORES)

    def din(name, shape, dtype=f32):
        return nc.dram_tensor(name, list(shape), dtype, kind="ExternalInput").ap()

    # ---- DRAM inputs (host-prepacked, per core) ----
    D_hbm = din("D", (B, 128, N))                 # gather data rows
    IDX_hbm = din("IDX", (B, NSUB, 128, GL // 16), i16)
    WT_hbm = din("WT", (128, ED), bf16)           # stacked K=21 edge-W
    WC_hbm = din("WC", (128, ED), bf16)           # stacked K=5 wrapped-d W
    W2A_hbm = din("W2A", (128, ED), bf16)
    W2B_hbm = din("W2B", (22, ED), bf16)
    B1_hbm = din("B1", (B, 128, 1))               # edge bias rows 0:128
    B1T_hbm = din("B1T", (B, 22, 1))              # edge bias rows 128:150
    B2_hbm = din("B2", (128, 1))
    B2T_hbm = din("B2T", (22, 1))
    VNP_hbm = din("VNP", (B, 3, NPC))             # node non_pos rows
    VBASE_hbm = din("VBASE", (B, 4, NPC))         # V[:,:,1:5] rows
    NW1NP_hbm = din("NW1NP", (3, ND))
    NW1A_hbm = din("NW1A", (128, ND))
    NW1B_hbm = din("NW1B", (22, ND))
    NW2_hbm = din("NW2", (ND, ND))
    NW3_hbm = din("NW3", (ND, ND))
    LW_hbm = din("LW", (ND, 4))
    THRH_hbm = din("THRH", (4, 1))                # [3,3,big,big]
    THRL_hbm = din("THRL", (4, 1))                # [-3,-3,-big,-big]
    NB1_hbm = din("NB1", (B, ND, 1))              # dt folded
    NB2_hbm = din("NB2", (ND, 1))
    NB3_hbm = din("NB3", (ND, 1))
    LB_hbm = din("LB", (4, 1))
    OUT_hbm = nc.dram_tensor("OUT", [B, 4, NPC], f32,
                             kind="ExternalOutput").ap()

    from contextlib import ExitStack
    with tile.TileContext(nc) as tc, ExitStack() as ctx:
        cpool = ctx.enter_context(tc.tile_pool(name="consts", bufs=1))
        dpool = ctx.enter_context(tc.tile_pool(name="dtile", bufs=2))
        ipool = ctx.enter_context(tc.tile_pool(name="idx", bufs=3))
        gpool = ctx.enter_context(tc.tile_pool(name="gath", bufs=2))
        spool1 = ctx.enter_context(tc.tile_pool(name="scr1", bufs=1))
        spool2 = ctx.enter_context(tc.tile_pool(name="scr2", bufs=1))
        spool3 = ctx.enter_context(tc.tile_pool(name="scr3", bufs=2))
        h1pool = ctx.enter_context(tc.tile_pool(name="h1", bufs=2))
        enpool = ctx.enter_context(tc.tile_pool(name="en", bufs=2))
        aggpool = ctx.enter_context(tc.tile_pool(name="agg", bufs=2))
        npool = ctx.enter_context(tc.tile_pool(name="node", bufs=2))
        ps1a = ctx.enter_context(tc.tile_pool(name="ps1a", bufs=1, space="PSUM"))
        ps1b = ctx.enter_context(tc.tile_pool(name="ps1b", bufs=1, space="PSUM"))
        ps2a = ctx.enter_context(tc.tile_pool(name="ps2a", bufs=1, space="PSUM"))
        ps2b = ctx.enter_context(tc.tile_pool(name="ps2b", bufs=1, space="PSUM"))

        # ---- load constants ----
        _cn = [0]

        def const(src, shape, dtype=f32):
            _cn[0] += 1
            t = cpool.tile(list(shape), dtype, tag=f"c{_cn[0]}")
            nc.sync.dma_start(t[...], src)
            return t

        WT = const(WT_hbm[...], (128, ED), bf16)
        WC = const(WC_hbm[...], (128, ED), bf16)
        W2A = const(W2A_hbm[...], (128, ED), bf16)
        W2B = const(W2B_hbm[...], (22, ED), bf16)
        B1 = [const(B1_hbm[b], (128, 1)) for b in range(B)]
        B1T = [const(B1T_hbm[b], (22, 1)) for b in range(B)]
        B2 = const(B2_hbm[...], (128, 1))
        B2T = const(B2T_hbm[...], (22, 1))
        NW1NP = const(NW1NP_hbm[...], (3, ND))
        NW1A = const(NW1A_hbm[...], (128, ND))
        NW1B = const(NW1B_hbm[...], (22, ND))
        NW2 = const(NW2_hbm[...], (ND, ND))
        NW3 = const(NW3_hbm[...], (ND, ND))
        LW = const(LW_hbm[...], (ND, 4))
        NB1 = [const(NB1_hbm[b], (ND, 1)) for b in range(B)]
        NB2 = const(NB2_hbm[...], (ND, 1))
        NB3 = const(NB3_hbm[...], (ND, 1))
        LB = const(LB_hbm[...], (4, 1))
        THRH = const(THRH_hbm[...], (4, 1))
        THRL = const(THRL_hbm[...], (4, 1))
        VNP = [const(VNP_hbm[b], (3, NPC)) for b in range(B)]
        VBASE = [const(VBASE_hbm[b], (4, NPC)) for b in range(B)]

        scS = spool1.tile([128, GL], f32, tag="scS")
        nc.vector.memset(scS[...], 0.0)

        gacc = None
        if variant == "gatheronly":
            gacc = npool.tile([4, NPC], f32, tag="gacc")
            nc.vector.memset(gacc[...], 0.0)

        for b in [bb for _ in range(repeat) for bb in range(B)]:
            Dt = dpool.tile([128, N], f32)
            nc.sync.dma_start(Dt[...], D_hbm[b])

            agg_a = aggpool.tile([128, NPC], f32, tag="agg_a")
            agg_b = aggpool.tile([22, NPC], f32, tag="agg_b")
            if variant == "gatheronly":
                nc.vector.memset(agg_a[...], 0.0)
                nc.vector.memset(agg_b[...], 0.0)

            for r in range(NSUB):
                idx = ipool.tile([128, GL // 16], i16)
                nc.sync.dma_start(idx[...], IDX_hbm[b, r])

                GT = gpool.tile([128, GL], f32)
                if variant != "nogather":
                    nc.gpsimd.ap_gather(
                        GT[...], Dt[...], idx[...],
                        channels=128, num_elems=N, d=1, num_idxs=GL,
                    )
                else:
                    nc.vector.tensor_copy(GT[...], Dt[:, 0:GL])
                if variant == "gatheronly":
                    nc.vector.tensor_add(gacc[...], gacc[...],
                                         GT[0:4, 0:NPC])
                    continue

                # periodic wrap, exact fp32: d_pre = src_pos - rcv_pos
                # (rows 32G+3,4; rcv rows DMA-shifted for alignment),
                # corr = (d_pre<=-3) - (d_pre>3), d = d_pre + 6*corr.
                # sc4b (bf16) rows 32G+3,4 carry the wrapped d for mm1.
                for G in range(4):
                    nc.sync.dma_start(scS[32 * G + 3:32 * G + 5, :],
                                      GT[32 * G + 19:32 * G + 21, :])
                sc1 = spool1.tile([128, GL], f32, tag="sc1")
                sc2 = spool2.tile([128, GL], f32)
                sc3 = spool3.tile([128, GL], f32, tag="sc3")
                sc4b = spool3.tile([128, GL], bf16, tag="sc4b")
                nc.vector.tensor_sub(sc1[...], GT[...], scS[...])
                nc.vector.tensor_scalar(sc2[...], sc1[...],
                                        HALF, None, ALU.is_gt)
                nc.vector.scalar_tensor_tensor(
                    sc3[...], sc1[...], -HALF, sc2[...],
                    ALU.is_le, ALU.subtract)
                nc.vector.scalar_tensor_tensor(
                    sc4b[...], sc3[...], BOX, sc1[...],
                    ALU.mult, ALU.add)
                GTb = gpool.tile([128, GL], bf16, tag="GTb")
                nc.vector.tensor_copy(GTb[...], GT[...])

                for G in range(4):
                    p0 = 32 * G
                    En_a = enpool.tile([128, GL], bf16, tag="en_a")
                    En_b = enpool.tile([22, GL], bf16, tag="en_b")
                    for nch in range(GL // NCH):
                        c0 = nch * NCH
                        p1a = ps1a.tile([128, NCH], f32, tag='ps1a')
                        p1b = ps1b.tile([22, NCH], f32, tag='ps1b')
                        for h in range(NCH // 512):
                            cs = slice(c0 + h * 512, c0 + (h + 1) * 512)
                            hs = slice(h * 512, (h + 1) * 512)
                            for (ps, ms) in ((p1a, slice(0, 128)),
                                             (p1b, slice(128, ED))):
                                nc.tensor.matmul(
                                    ps[:, hs], lhsT=WT[p0:p0 + 21, ms],
                                    rhs=GTb[p0:p0 + 21, cs],
                                    start=True, stop=False,
                                    tile_position=(p0, 0))
                                nc.tensor.matmul(
                                    ps[:, hs], lhsT=WC[p0:p0 + 5, ms],
                                    rhs=sc4b[p0:p0 + 5, cs],
                                    start=False, stop=True,
                                    tile_position=(p0, 0))
                        h1a = h1pool.tile([128, NCH], bf16, tag="h1a")
                        h1b = h1pool.tile([22, NCH], bf16, tag="h1b")
                        nc.scalar.activation(h1a[...], p1a[...], AF.Relu,
                                             bias=B1[b][...])
                        nc.scalar.activation(h1b[...], p1b[...], AF.Relu,
                                             bias=B1T[b][...])
                        p2a = ps2a.tile([128, NCH], f32, tag='ps2a')
                        p2b = ps2b.tile([22, NCH], f32, tag='ps2b')
                        for h in range(NCH // 512):
                            hs = slice(h * 512, (h + 1) * 512)
                            for (ps, ms) in ((p2a, slice(0, 128)),
                                             (p2b, slice(128, ED))):
                                nc.tensor.matmul(
                                    ps[:, hs], lhsT=W2A[:, ms],
                                    rhs=h1a[:, hs], start=True, stop=False)
                                nc.tensor.matmul(
                                    ps[:, hs], lhsT=W2B[:, ms],
                                    rhs=h1b[:, hs], start=False, stop=True)
                        nc.scalar.activation(En_a[:, c0:c0 + NCH], p2a[...],
                                             AF.Relu, bias=B2[0:128, :])
                        nc.scalar.activation(En_b[:, c0:c0 + NCH], p2b[...],
                                             AF.Relu, bias=B2T[...])
                    n0 = 128 * G + 32 * r
                    nc.vector.tensor_reduce(
                        agg_a[:, n0:n0 + 32],
                        En_a[...].rearrange("p (n s) -> p n s", s=64),
                        mybir.AxisListType.X, ALU.add)
                    nc.vector.tensor_reduce(
                        agg_b[:, n0:n0 + 32],
                        En_b[...].rearrange("p (n s) -> p n s", s=64),
                        mybir.AxisListType.X, ALU.add)

            # ---- node MLP ----
            pn = ps1a.tile([ND, NPC], f32, tag='ps1a')
            nc.tensor.matmul(pn[...], lhsT=NW1NP[...], rhs=VNP[b][...],
                             start=True, stop=False)
            nc.tensor.matmul(pn[...], lhsT=NW1A[...], rhs=agg_a[...],
                             start=False, stop=False)
            nc.tensor.matmul(pn[...], lhsT=NW1B[...], rhs=agg_b[...],
                             start=False, stop=True)
            hn1 = npool.tile([ND, NPC], f32, tag="hn")
            nc.vector.tensor_scalar(hn1[...], pn[...], NB1[b][...], 0.0,
                                    ALU.add, ALU.max)
            pn2 = ps2a.tile([ND, NPC], f32, tag='ps2a')
            nc.tensor.matmul(pn2[...], lhsT=NW2[...], rhs=hn1[...],
                             start=True, stop=True)
            hn2 = npool.tile([ND, NPC], f32, tag="hn")
            nc.vector.tensor_scalar(hn2[...], pn2[...], NB2[...], 0.0,
                                    ALU.add, ALU.max)
            pn3 = ps1b.tile([ND, NPC], f32, tag='ps1b')
            nc.tensor.matmul(pn3[...], lhsT=NW3[...], rhs=hn2[...],
                             start=True, stop=True)
            hn3 = npool.tile([ND, NPC], f32, tag="hn")
            nc.vector.tensor_scalar(hn3[...], pn3[...], NB3[...], 0.0,
                                    ALU.add, ALU.max)
            pf = ps2b.tile([4, NPC], f32, tag='ps2b')
            nc.tensor.matmul(pf[...], lhsT=LW[...], rhs=hn3[...],
                             start=True, stop=True)
            delta = npool.tile([4, NPC], f32, tag="delta")
            nc.scalar.activation(delta[...], pf[...], AF.Identity,
                                 bias=LB[...])
            newv = npool.tile([4, NPC], f32, tag="newv")
            nc.vector.tensor_add(newv[...], delta[...], VBASE[b][...])
            tA = npool.tile([4, NPC], f32, tag="tA")
            nc.vector.tensor_scalar(tA[...], newv[...], THRH[...], None,
                                    ALU.is_ge)
            tB = npool.tile([4, NPC], f32, tag="tB")
            nc.vector.scalar_tensor_tensor(tB[...], newv[...], THRL[...],
                                           tA[...], ALU.is_lt, ALU.subtract)
            outc = npool.tile([4, NPC], f32, tag="outc")
            nc.vector.scalar_tensor_tensor(outc[...], tB[...], BOX,
                                           newv[...], ALU.mult, ALU.add)
            if variant == "gatheronly":
                nc.vector.tensor_add(outc[...], outc[...], gacc[...])
            nc.sync.dma_start(OUT_hbm[b], outc[...])

    nc.compile()
    return nc


def _pack_inputs(V, dt, eW1, eb1, eW2, eb2, nW1, nb1, nW2, nb2, nW3, nb3,
                 lW, lb, R_s, R_r):
    """Build the per-core input dicts (host-side preprocessing)."""
    V = np.asarray(V, np.float32)
    dt = np.asarray(dt, np.float32)
    eW1 = np.asarray(eW1, np.float32)
    eW2 = np.asarray(eW2, np.float32)
    nW1 = np.asarray(nW1, np.float32)
    R_s = np.asarray(R_s).astype(np.int64)
    R_r = np.asarray(R_r).astype(np.int64)

    import ml_dtypes
    bfl = ml_dtypes.bfloat16
    fmap = [0, 3, 4, 1, 2] + [0] * 11
    Vt = np.transpose(V, (0, 2, 1))                    # [B, 5, N]
    D = np.empty((B, 128, N), np.float32)
    for s in range(8):                                  # 8 sub-groups of 16
        D[:, 16 * s:16 * (s + 1), :] = Vt[:, fmap, :]

    # W~ [21 rows] stacked at the four 32-partition bases
    WTs = np.zeros((128, ED), np.float32)
    WCs = np.zeros((128, ED), np.float32)
    w21 = np.zeros((21, ED), np.float32)
    w21[0:3] = eW1[0:3]
    w21[16:19] = eW1[3:6]
    for G in range(4):
        WTs[32 * G:32 * G + 21] = w21
        WCs[32 * G + 3] = eW1[6]
        WCs[32 * G + 4] = eW1[7]
    WTs = WTs.astype(bfl)
    WCs = WCs.astype(bfl)

    W2A = np.ascontiguousarray(eW2[0:128]).astype(bfl)
    W2B = np.ascontiguousarray(eW2[128:150]).astype(bfl)
    B1 = np.zeros((B, 128, 1), np.float32)
    B1T = np.zeros((B, 22, 1), np.float32)
    for b in range(B):
        bb = np.asarray(eb1, np.float32) + dt[b, 0] * eW1[8]
        B1[b, :, 0] = bb[0:128]
        B1T[b, :, 0] = bb[128:150]
    eb2f = np.asarray(eb2, np.float32)
    B2 = np.ascontiguousarray(eb2f[0:128]).reshape(128, 1)
    B2T = np.ascontiguousarray(eb2f[128:150]).reshape(22, 1)

    NW1NP = np.ascontiguousarray(nW1[0:3])
    NW1A = np.ascontiguousarray(nW1[3:131])
    NW1B = np.ascontiguousarray(nW1[131:153])
    NB1 = np.zeros((B, ND, 1), np.float32)
    for b in range(B):
        NB1[b, :, 0] = np.asarray(nb1, np.float32) + dt[b, 0] * nW1[153]
    BIG = np.float32(1e30)
    THRH = np.array([[HALF], [HALF], [BIG], [BIG]], np.float32)
    THRL = np.array([[-HALF], [-HALF], [-BIG], [-BIG]], np.float32)
    NB2 = np.asarray(nb2, np.float32).reshape(ND, 1)
    NB3 = np.asarray(nb3, np.float32).reshape(ND, 1)
    LB = np.asarray(lb, np.float32).reshape(4, 1)
    NW2 = np.asarray(nW2, np.float32)
    NW3 = np.asarray(nW3, np.float32)
    LW = np.asarray(lW, np.float32)

    in_maps = []
    for c in range(NCORES):
        nodes = slice(NPC * c, NPC * (c + 1))
        IDX = np.empty((B, NSUB, 128, GL // 16), np.int16)
        for b in range(B):
            for r in range(NSUB):
                for G in range(4):
                    e0 = EPC * c + SUBE * G + GL * r
                    ls = R_s[b, e0:e0 + GL].astype(np.int16)
                    lr = R_r[b, e0:e0 + GL].astype(np.int16)
                    IDX[b, r, 32 * G:32 * G + 16] = \
                        ls.reshape(GL // 16, 16).T
                    IDX[b, r, 32 * G + 16:32 * G + 32] = \
                        lr.reshape(GL // 16, 16).T
        VNP = np.ascontiguousarray(
            np.transpose(V[:, nodes][:, :, [0, 3, 4]], (0, 2, 1)))
        VBASE = np.ascontiguousarray(
            np.transpose(V[:, nodes][:, :, 1:5], (0, 2, 1)))
        in_maps.append({
            "D": D, "IDX": IDX, "WT": WTs, "WC": WCs,
            "W2A": W2A, "W2B": W2B, "B1": B1, "B1T": B1T,
            "B2": B2, "B2T": B2T,
            "VNP": VNP, "VBASE": VBASE,
            "NW1NP": NW1NP, "NW1A": NW1A, "NW1B": NW1B,
            "NW2": NW2, "NW3": NW3, "LW": LW,
            "NB1": NB1, "NB2": NB2, "NB3": NB3, "LB": LB,
            "THRH": THRH, "THRL": THRL,
        })
    return in_maps


def _get_runner():
    """Build (once) a cached jitted PJRT runner for the compiled program."""
    if "runner" in _CACHE:
        return _CACHE["runner"]
    if "nc" not in _CACHE:
        _CACHE["nc"] = _build_program()
    _CACHE["runner"] = _make_runner(_CACHE["nc"])
    return _CACHE["runner"]


def _make_runner(nc):
    import jax
    from jax.sharding import Mesh, PartitionSpec
    from jax.experimental.shard_map import shard_map
    from concourse import bass2jax, mybir

    bass2jax.install_neuronx_cc_hook()

    part_name = (nc.partition_id_tensor.name
                 if nc.partition_id_tensor else None)
    in_names, out_names, out_avals, zero_outs = [], [], [], []
    for alloc in nc.m.functions[0].allocations:
        if not isinstance(alloc, mybir.MemoryLocationSet):
            continue
        name = alloc.memorylocations[0].name
        if alloc.kind == "ExternalInput":
            if name != part_name:
                in_names.append(name)
        elif alloc.kind == "ExternalOutput":
            shape = tuple(alloc.tensor_shape)
            dtype = mybir.dt.np(alloc.dtype)
            out_names.append(name)
            out_avals.append(jax.core.ShapedArray(shape, dtype))
            zero_outs.append(np.zeros(shape, dtype))
    n_params = len(in_names)
    all_in_names = in_names + out_names
    if part_name is not None:
        all_in_names = all_in_names + [part_name]

    def _body(*args):
        operands = list(args)
        if part_name is not None:
            operands.append(bass2jax.partition_id_tensor())
        outs = bass2jax._bass_exec_p.bind(
            *operands,
            out_avals=tuple(out_avals),
            in_names=tuple(all_in_names),
            out_names=tuple(out_names),
            lowering_input_output_aliases=(),
            sim_require_finite=True,
            sim_require_nnan=True,
            nc=nc,
        )
        return tuple(outs)

    devices = jax.devices()[:NCORES]
    mesh = Mesh(np.asarray(devices), ("core",))
    n_outs = len(out_names)
    sharded = jax.jit(
        shard_map(
            _body, mesh=mesh,
            in_specs=(PartitionSpec("core"),) * (n_params + n_outs),
            out_specs=(PartitionSpec("core"),) * n_outs,
            check_rep=False,
        ),
        keep_unused=True,
    )
    return {
        "fn": sharded, "in_names": in_names, "out_names": out_names,
        "out_avals": out_avals, "zero_outs": zero_outs, "mesh": mesh,
    }


def _run(in_maps):
    rn = _get_runner()
    concat_in = [
        np.concatenate([m[name] for m in in_maps], axis=0)
        for name in rn["in_names"]
    ]
    concat_zeros = [
        np.zeros((NCORES * z.shape[0], *z.shape[1:]), z.dtype)
        for z in rn["zero_outs"]
    ]
    out_arrs = rn["fn"](*concat_in, *concat_zeros)
    return out_arrs


def kernel(**inputs):
    in_maps = _pack_inputs(**inputs)
    out_arrs = _run(in_maps)
    rn = _CACHE["runner"]
    i = rn["out_names"].index("OUT")
    o = np.asarray(out_arrs[i]).reshape(NCORES, B, 4, NPC)
    out = np.empty((B, N, 4), np.float32)
    for c in range(NCORES):
        out[:, NPC * c:NPC * (c + 1), :] = np.transpose(o[c], (0, 2, 1))
    return out
